# revision 44
# baseline (speedup 1.0000x reference)
"""DiffusionGraphConv Trainium2 kernel (8-core SPMD, data-parallel over batch).

Math (per reference):
  x = concat(inputs, state)           -> [B, N, F]   B=32, N=4096, F=128
  x0 = x transposed to [N, F*B]
  per support s (2): x1 = A_s x0 ; x2 = 2 A_s x1 - x0   (A_s dense from COO)
  out[b*N+n, o] = sum_{f,m} xs_m[n, f, b] * W[f*M+m, o] + bias[o]

Sharding: batch across 8 cores (4 batches/core, C = 4*F = 512 columns of x0).

Device algorithm per core, fp8 (e4m3) DoubleRow matmuls for both A
applications (2 k-tiles contracted per pass = 2x PE throughput):
  Phase 1 (per support): psum = (8*A_s) @ X0_f8; x1sb = 0.25*psum = 2*A_s X0,
     stored fp8.  (a1 holds 8*A^T in fp8; X0_f8 is fp8 x0.)
  Phase 2 (per 512-wide output-node chunk rc):
     p2 chunk   : (A_s @ x1sb)^T via DoubleRow (lhsT = x1sb fp8 pairs,
                  rhs = a2 = A^T fp8 pairs) = (x2 + x0)^T, staged f32r
     x1T chunks : PE transpose of fp8 x1sb tiles -> f32r staging
     out^T[o, chunk] = sum_m V_m^T @ xmT_chunk   (5 fp32r matmuls, PSUM acc)
     (+ bias via ACT on evacuation)
  x0^T comes pre-transposed from the host (f32r, SBUF-resident), so the m0
  term needs no device transposes.  V folding kills the x2 subtraction:
  V0' = W0 - W2 - W4 (the +x0 inside p2 cancels), V1 = W1/2, V3 = W3/2
  (x1sb stores 2*x1), V2 = W2, V4 = W4.

Everything is hardcoded for the reference shapes; host does only layout prep
(dense-ify supports, transpose/shard x0, tile A, fp8 casts) and reassembly.
"""

import numpy as np
import ml_dtypes

import concourse.bass as bass
import concourse.tile as tile
from concourse import bacc, mybir
from concourse import bass_utils

B, N, D, H, O, S = 32, 4096, 64, 64, 128, 2
F = D + H                    # 128
NCORES = 8
BLOC = B // NCORES           # 4 batches per core
C = BLOC * F                 # 512 columns per core
NBLK = N // 128              # 32 n-tiles
NRC = N // 512               # 8 output-node chunks
M = 5
NPAIR = NBLK // 2            # 16 DoubleRow pairs over the contraction

F32 = mybir.dt.float32
F32R = mybir.dt.float32r
BF16 = mybir.dt.bfloat16
FP8 = mybir.dt.float8e4
DR = mybir.MatmulPerfMode.DoubleRow
NPF8 = ml_dtypes.float8_e4m3

_CACHE = {}


def build_nc():
    nc = bacc.Bacc("TRN2", target_bir_lowering=False, debug=False)

    # ---- DRAM tensors ----
    # x0f8[p, t, c] = x0_core[t*128+p, c], c = j*128+f  (fp8, partition-major)
    x0f8_d = nc.dram_tensor("x0f8", [128, NBLK, C], FP8, kind="ExternalInput")
    # x0t[j, f, n] = x0_core[n, j*128+f]  (f32, host-pre-transposed)
    x0t_d = nc.dram_tensor("x0t", [BLOC, 128, N], F32R, kind="ExternalInput")
    # a1[s, i, p, k, q] = 8*AT_s[k*128+p, i*128+q]  (fp8)
    a1_d = nc.dram_tensor("a1", [S, NBLK, 128, NBLK, 128], FP8,
                          kind="ExternalInput")
    # a2[s, rc, p, k, q] = AT_s[k*128+p, rc*512+q]  (fp8; k-major for pairs)
    a2_d = nc.dram_tensor("a2", [S, NRC, 128, NBLK, 512], FP8,
                          kind="ExternalInput")
    # weights: v0 = W0-W2-W4 (f32r); v13 = fp8 {W1/2, W3/2} and
    # v24 = fp8 {W2, W4} pairs for DoubleRow-fused output matmuls
    v_d = nc.dram_tensor("v", [1, 128, 128], F32R, kind="ExternalInput")
    v13_d = nc.dram_tensor("v13", [128, 2, 128], FP8, kind="ExternalInput")
    v24_d = nc.dram_tensor("v24", [128, 2, 128], FP8, kind="ExternalInput")
    bias_d = nc.dram_tensor("bias", [128, 1], F32, kind="ExternalInput")
    idn8_d = nc.dram_tensor("idn8", [128, 128], FP8, kind="ExternalInput")
    # output: out[o, j, n]  (partition-major to match the SBUF staging tile)
    out_d = nc.dram_tensor("out", [128, BLOC, N], F32, kind="ExternalOutput")

    A1H = 16     # k-tiles per a1 half-slab DMA
    NH = NBLK // A1H

    with tile.TileContext(nc) as tc:
        with (
            tc.tile_pool(name="big", bufs=1) as big,
            tc.tile_pool(name="a1p", bufs=6) as a1p,
            tc.tile_pool(name="a2p", bufs=6) as a2p,
            tc.tile_pool(name="stg", bufs=1) as stg,
            tc.tile_pool(name="psp", bufs=1, space=bass.MemorySpace.PSUM) as psp,
        ):
            # ---- load resident tensors ----
            # x0f8 in chunked tiles so phase-1 matmuls start as soon as the
            # first 256KB lands (tile-granular deps); first quarter is split
            # into two eighths to cut the startup wait further
            x0q8 = [big.tile([128, 4, C], FP8, tag="x0e8_0", name="x0e8_0"),
                    big.tile([128, 4, C], FP8, tag="x0e8_1", name="x0e8_1")]
            x0q8 += [big.tile([128, 8, C], FP8, tag=f"x0q8_{qq}",
                              name=f"x0q8_{qq}")
                     for qq in range(1, 4)]

            def x0pair(hp):  # [128, 2, C] k-tile pair (2hp, 2hp+1)
                if hp < 4:
                    return x0q8[hp // 2][:, 2 * (hp % 2):2 * (hp % 2) + 2, :]
                return x0q8[hp // 4 + 1][:, 2 * (hp % 4):2 * (hp % 4) + 2, :]

            x0t = big.tile([128, BLOC, N], F32R, tag="x0t")

            def fetch_a1(s, i, nchunks=NH, tag="a1", bufs=None):
                hs = []
                step = NBLK // nchunks
                for h in range(nchunks):
                    ah = a1p.tile([128, step, 128], FP8, tag=tag,
                                  name=f"a1_{s}_{i}_{h}",
                                  bufs=bufs or nchunks * 4)
                    nc.sync.dma_start(
                        ah[:], a1_d[s, i, :, h * step:(h + 1) * step, :])
                    hs.append(ah)
                return hs

            # first a1 slab in small chunks (own tag: keeps the steady-state
            # "a1" pool accounting intact) + x0 quarters first, so the first
            # matmul's dependencies land as early as possible
            a1_pre = {(0, 0): fetch_a1(0, 0, nchunks=8, tag="a1s", bufs=8)}
            nc.sync.dma_start(x0q8[0][:], x0f8_d[:, 0:4, :])
            nc.sync.dma_start(x0q8[1][:], x0f8_d[:, 4:8, :])
            nc.sync.dma_start(x0q8[2][:], x0f8_d[:, 8:16, :])
            a1_pre[(0, 1)] = fetch_a1(0, 1)
            for qq in range(3, 5):
                nc.sync.dma_start(x0q8[qq][:],
                                  x0f8_d[:, 8 * (qq - 1):8 * qq, :])
            a1_pre[(0, 2)] = fetch_a1(0, 2)
            vsb = big.tile([128, 1, 128], F32R, tag="v")
            nc.sync.dma_start(vsb[:, 0, :], v_d[0])
            v13 = big.tile([128, 2, 128], FP8, tag="v13")
            nc.sync.dma_start(v13[:], v13_d[:])
            v24 = big.tile([128, 2, 128], FP8, tag="v24")
            nc.sync.dma_start(v24[:], v24_d[:])
            bias_sb = big.tile([128, 1], F32, tag="bias")
            nc.sync.dma_start(bias_sb[:], bias_d[:])
            idn8 = big.tile([128, 128], FP8, tag="idn8")
            nc.sync.dma_start(idn8[:], idn8_d[:])

            # ---- phase 1: x1sb[s] = 2 * A_s @ X0  (fp8 out) ----
            # x0t (8.4MB, needed only in phase 2) is drip-loaded between
            # early phase-1 iterations so it never blocks the a1 stream
            x1sb = []
            for s in range(S):
                x1 = big.tile([128, NBLK, C], FP8, tag=f"x1_{s}")
                x1sb.append(x1)
                for i in range(NBLK):
                    halves = a1_pre.pop((s, i), None) or fetch_a1(s, i)
                    # drip-load x0t in 512KB chunks so it never delays the
                    # a1 stream in the DMA queues
                    if s == 0 and 3 <= i < 19:
                        ch = i - 3
                        jj, hh = ch // 4, ch % 4
                        nc.sync.dma_start(
                            x0t[:, jj, hh * 1024:(hh + 1) * 1024],
                            x0t_d[jj, :, hh * 1024:(hh + 1) * 1024])
                    ps = psp.tile([128, C], F32, tag=f"rot{i % 2}",
                                  name=f"acc1_{s}_{i}")
                    step = NBLK // len(halves)
                    for hp in range(NPAIR):
                        q = halves[(2 * hp) // step]
                        w = (2 * hp) % step
                        nc.tensor.matmul(
                            ps[:],
                            q[:, w:w + 2, :],
                            x0pair(hp),
                            start=(hp == 0),
                            stop=(hp == NPAIR - 1),
                            perf_mode=DR,
                        )
                    nc.scalar.mul(x1[:, i, :], ps[:], 0.25)

            # ---- phase 2: per output-node chunk rc ----
            for rc in range(NRC):
                # p2 chunks: (2 A_s x1)^T = (x2 + x0)^T via DoubleRow
                p2 = {}
                for s in range(S):
                    # rotate over 5 PSUM tags so the s=1 matmuls never wait
                    # on the s=0 evacuation casts (spare-bank rotation)
                    pxj = [psp.tile([128, 512], F32,
                                    tag=f"rot{(s * BLOC + j) % 5}",
                                    name=f"px2_{s}_{rc}_{j}")
                           for j in range(BLOC)]
                    for hq in range(NPAIR // 2):
                        a2t = a2p.tile([128, 4, 512], FP8, tag="a2")
                        nc.sync.dma_start(
                            a2t[:], a2_d[s, rc, :, 4 * hq:4 * hq + 4, :])
                        for half in range(2):
                            hp = 2 * hq + half
                            for j in range(BLOC):
                                nc.tensor.matmul(
                                    pxj[j][:],
                                    x1sb[s][:, 2 * hp:2 * hp + 2,
                                            j * 128:(j + 1) * 128],
                                    a2t[:, 2 * half:2 * half + 2, :],
                                    start=(hp == 0),
                                    stop=(hp == NPAIR - 1),
                                    perf_mode=DR,
                                )
                    for j in range(BLOC):
                        if s == 0:
                            p2[j] = stg.tile([128, 2, 512], FP8, tag="p2",
                                             bufs=8, name=f"p2_{rc}_{j}")
                        nc.vector.tensor_copy(p2[j][:, s, :], pxj[j][:])

                # out^T accumulation: batch all transposes/copies first so
                # the po matmuls (issued later on the in-order PE) never
                # wait on a copy that was issued just before them
                st12s = []
                for j in range(BLOC):
                    # both supports' x1^T staged fp8 into one pair tile for
                    # the DoubleRow-fused m1+m3 matmul
                    st12 = stg.tile([128, 2, 512], FP8, tag="x1t", bufs=4)
                    for s in range(S):
                        # fp8 transpose writes 16-bit PSUM lanes: element
                        # step 2, so view the tile as [128, 512, 2] fp8
                        ptr = psp.tile([128, 512, 2], FP8, tag="trx", bufs=2)
                        for t in range(4):
                            nt = rc * 4 + t
                            nc.tensor.transpose(
                                ptr[:, t * 128:(t + 1) * 128, 0],
                                x1sb[s][:, nt, j * 128:(j + 1) * 128],
                                idn8[:])
                        if (j + s) % 2 == 0:
                            nc.scalar.copy(st12[:, s, :], ptr[:, :, 0])
                        else:
                            nc.vector.tensor_copy(st12[:, s, :], ptr[:, :, 0])
                    st12s.append(st12)

                for j in range(BLOC):
                    st12 = st12s[j]
                    po = psp.tile([128, 512], F32,
                                  tag=f"rot{(2 * BLOC + j) % 5}",
                                  name=f"po_{rc}_{j}")
                    nc.tensor.matmul(
                        po[:], vsb[:, 0, :],
                        x0t[:, j, rc * 512:(rc + 1) * 512],
                        start=True, stop=False)
                    nc.tensor.matmul(
                        po[:], v13[:], st12[:],
                        start=False, stop=False, perf_mode=DR)
                    nc.tensor.matmul(
                        po[:], v24[:], p2[j][:],
                        start=False, stop=True, perf_mode=DR)
                    ot = stg.tile([128, 512], F32, tag="ot", bufs=4)
                    if j % 2 == 0:
                        nc.scalar.add(ot[:], po[:], bias_sb[:, 0:1])
                    else:
                        nc.vector.tensor_scalar_add(
                            ot[:], po[:], bias_sb[:, 0:1])
                    nc.sync.dma_start(
                        out_d[:, j, rc * 512:(rc + 1) * 512], ot[:])

    nc.compile()
    return nc


def _prep_shared(sup_rows, sup_cols, sup_vals, weight, biases):
    AT = np.zeros((S, N, N), dtype=np.float32)
    for s in range(S):
        np.add.at(AT[s], (sup_cols[s].astype(np.int64),
                          sup_rows[s].astype(np.int64)),
                  sup_vals[s].astype(np.float32))
    # a1[s, i, p, k, q] = 8*AT[s][k*128+p, i*128+q]
    a1 = np.ascontiguousarray(
        (AT * 8.0).reshape(S, NBLK, 128, NBLK, 128).transpose(0, 3, 2, 1, 4)
    ).astype(NPF8)
    # a2[s, rc, p, k, q] = AT[s][k*128+p, rc*512+q]
    a2 = np.ascontiguousarray(
        AT.reshape(S, NBLK, 128, NRC, 512).transpose(0, 3, 2, 1, 4)
    ).astype(NPF8)

    Wm = np.asarray(weight, dtype=np.float32).reshape(F, M, O)
    V = np.ascontiguousarray(
        (Wm[:, 0, :] - Wm[:, 2, :] - Wm[:, 4, :])[None]).astype(np.float32)
    v13 = np.ascontiguousarray(
        np.stack([Wm[:, 1, :] * 0.5, Wm[:, 3, :] * 0.5], axis=1)
    ).astype(NPF8)
    v24 = np.ascontiguousarray(
        np.stack([Wm[:, 2, :], Wm[:, 4, :]], axis=1)).astype(NPF8)
    bias = np.asarray(biases, dtype=np.float32).reshape(128, 1)
    idn8 = np.eye(128, dtype=NPF8)
    return a1, a2, V, v13, v24, bias, idn8


def kernel(inputs, state, sup_rows, sup_cols, sup_vals, weight, biases,
           output_size=128, **_ignored):
    inputs = np.asarray(inputs, dtype=np.float32)
    state = np.asarray(state, dtype=np.float32)
    x = np.concatenate(
        [inputs.reshape(B, N, D), state.reshape(B, N, H)], axis=2)  # [B,N,F]

    a1, a2, V, v13, v24, bias, idn8 = _prep_shared(
        np.asarray(sup_rows), np.asarray(sup_cols), np.asarray(sup_vals),
        weight, biases)

    if "nc" not in _CACHE:
        _CACHE["nc"] = build_nc()
    nc = _CACHE["nc"]

    in_maps = []
    for c in range(NCORES):
        xb = x[c * BLOC:(c + 1) * BLOC]                       # [BLOC, N, F]
        # x0f8[p, t, c]: x0_core[t*128+p, j*128+f] = xb[j, t*128+p, f]
        xc = np.ascontiguousarray(
            xb.transpose(1, 0, 2).reshape(NBLK, 128, C).transpose(1, 0, 2))
        xt = np.ascontiguousarray(xb.transpose(0, 2, 1))      # [BLOC, F, N]
        in_maps.append({
            "x0f8": xc.astype(NPF8), "x0t": xt,
            "a1": a1, "a2": a2, "v": V, "v13": v13, "v24": v24,
            "bias": bias, "idn8": idn8,
        })

    res = None
    for attempt in range(3):
        try:
            res = bass_utils.run_bass_kernel_spmd(
                nc, in_maps, core_ids=list(range(NCORES)), trace=False)
            break
        except Exception:
            if attempt == 2:
                raise
            import time as _time
            _time.sleep(15 * (attempt + 1))

    # reassemble: out_core[o, j, n] -> out[b, n, o]
    outs = np.stack([res.results[c]["out"] for c in range(NCORES)])
    full = outs.transpose(0, 2, 3, 1).reshape(B, N, O)
    return np.ascontiguousarray(full.reshape(B, N * O))
